# revision 3
# baseline (speedup 1.0000x reference)
"""Multi-head attention block on 8 Trainium2 NeuronCores — v3.

Problem: B=4, N=2048, C=768, H=12, HD=64 (f32).
  qkv = x @ w_qkv + b_qkv ; attn = softmax(q*k^T/8) ; out = (attn@v) @ w_proj + b_proj

Sharding: data-parallel over batch (4) x tensor-parallel over heads (2 groups
of 6 heads). Core c handles batch c//2, head-group c%2; the host sums the two
head-group partials per batch in f32 and adds b_proj.

v3 design (on top of v2): row-tiled score PAIRS. The two heads of a pair sit
at PE row groups 0-63 / 64-127 (tile_position auto-derived from the lhsT/out
base partitions), so their K=64 score matmuls execute CONCURRENTLY in the PE
array — ~2x on the scores phase. Attention processes a head PAIR per stream:
  - scores for 3 kt x 2 heads accumulate into a 3-bank psum tile
    [128, 3072] (pair-interleaved 512-col tiles), exp'd by ONE Act op.
  - AV accumulates both heads into a 2-bank [128, 1024] psum tile.
  - av is drained to SBUF immediately (frees the banks for the next pair),
    then normalized from SBUF: exact reciprocal (custom-DVE ops corrupt on
    HW) + broadcast + multiply.
q^T compute for chunk j+1 is emitted inside attention(j) to keep the PE fed
while Act drains the exp backlog.
"""

import numpy as np

from concourse import bacc, bass, bass_utils, tile
from concourse import mybir

B, N, C, H, HD = 4, 2048, 768, 12, 64
SCALE = HD ** -0.5
P = 128
QC = 512              # q-chunk (free dim per matmul / psum bank)
NT = N // P           # 16 key tiles
CT = C // P           # 6 contraction tiles over C
NCH = N // QC         # 4 q-chunks
HPC = 6               # heads per core
CQK = HPC * HD        # 384
VW = 65               # V columns per head incl. ones column
F32 = mybir.dt.float32
BF16 = mybir.dt.bfloat16
EXP = mybir.ActivationFunctionType.Exp
COPY = mybir.ActivationFunctionType.Copy

# kt-group sizes per (j, pair): each group's 2*g score tiles (both heads,
# pair-interleaved) go to one psum tile and one Act exp. The (2,1) alternation
# maps 2-kt groups to the 4-bank psA and 1-kt groups to the 2-bank psB
# (4 + 2 + 2 av = 8 banks).
GROUPS = (2, 1, 2, 1, 2, 1, 2, 1, 2, 1, 1)

_CACHE = {}


def build_program(mm_dt=BF16, bcast_mode="pool"):
    MMDT = mm_dt
    nc = bacc.Bacc("TRN2", target_bir_lowering=False, debug=False, num_devices=8)

    xt_d = nc.dram_tensor("xt", [C, N], MMDT, kind="ExternalInput")
    w_d = nc.dram_tensor("w", [C, 3 * CQK], MMDT, kind="ExternalInput")
    wp_d = nc.dram_tensor("wp", [CQK, C], MMDT, kind="ExternalInput")
    bqk_d = nc.dram_tensor("bqk", [P, CT], F32, kind="ExternalInput")
    bv_d = nc.dram_tensor("bv", [1, CQK], MMDT, kind="ExternalInput")
    out_d = nc.dram_tensor("out", [N, C], MMDT, kind="ExternalOutput")

    with tile.TileContext(nc) as tc, nc.allow_low_precision(
            reason="bf16 matmuls; host accumulates partials in f32"):
        with (
            tc.tile_pool(name="const", bufs=1) as cpool,
            tc.tile_pool(name="persist", bufs=1) as pp,
        ):
            ones = cpool.tile([1, P], MMDT, name="ones", tag="ones")
            nc.gpsimd.memset(ones[:], 1.0)
            ones_f = cpool.tile([P, NT * HPC], F32, name="ones_f", tag="ones_f")
            nc.gpsimd.memset(ones_f[:], 1.0)
            bqk = cpool.tile([P, CT], F32, name="bqk", tag="bqk")
            nc.sync.dma_start(bqk[:], bqk_d[:])
            bv = cpool.tile([1, CQK], MMDT, name="bv", tag="bv")
            nc.sync.dma_start(bv[:], bv_d[:])

            xt_sb = []
            for ct in range(CT):
                t = pp.tile([P, N], MMDT, name=f"xt{ct}", tag=f"xt{ct}")
                xt_sb.append(t)
            for j in range(NCH):
                for ct in range(CT):
                    nc.sync.dma_start(
                        xt_sb[ct][:, j * QC:(j + 1) * QC],
                        xt_d[ct * P:(ct + 1) * P, j * QC:(j + 1) * QC])
            w_sb = []
            for ct in range(CT):
                t = pp.tile([P, 3 * CQK], MMDT, name=f"w{ct}", tag=f"w{ct}")
                for h in range(3):
                    nc.sync.dma_start(
                        t[:, h * CQK:(h + 1) * CQK],
                        w_d[ct * P:(ct + 1) * P, h * CQK:(h + 1) * CQK])
                w_sb.append(t)
            wp_sb = []
            for p in range(3):
                t = pp.tile([P, C], MMDT, name=f"wp{p}", tag=f"wp{p}")
                nc.sync.dma_start(t[:], wp_d[p * P:(p + 1) * P, :])
                wp_sb.append(t)

            qT = [pp.tile([P, N], MMDT, name=f"q{p}", tag=f"q{p}") for p in range(3)]
            kT = [pp.tile([P, N], MMDT, name=f"k{p}", tag=f"k{p}") for p in range(3)]
            v_sb = pp.tile([P, NT * HPC * VW], MMDT, name="v", tag="v")
            v_view = v_sb[:].rearrange("p (t w) -> p t w", w=VW)
            nc.vector.tensor_copy(
                v_view[:, :, HD:HD + 1],
                ones_f[:, 0:NT * HPC].rearrange("p (t w) -> p t w", w=1))
            # o^T packed per head-pair: rows 0-63 head 2p, 64-127 head 2p+1
            o_sb = [pp.tile([P, N], MMDT, name=f"o{p}", tag=f"o{p}") for p in range(3)]

            # ---------------- phase 1: k^T, v, q^T ----------------
            with (
                tc.tile_pool(name="exsb", bufs=2) as ex_pool,
                tc.tile_pool(name="avsb", bufs=2) as avs_pool,
                tc.tile_pool(name="recs", bufs=2) as rec_pool,
                tc.tile_pool(name="bcs", bufs=2) as bc_pool,
            ):
                with (
                    tc.tile_pool(name="qkps", bufs=3, space="PSUM") as qkps,
                    tc.tile_pool(name="vps", bufs=2, space="PSUM") as vpsp,
                ):
                    def qk_cols(j, colts):
                        for colt in colts:
                            qps = qkps.tile([P, QC], F32, name="qkp", tag="qkp")
                            for ct in range(CT):
                                nc.tensor.matmul(
                                    qps[:],
                                    w_sb[ct][:, colt * P:(colt + 1) * P],
                                    xt_sb[ct][:, j * QC:(j + 1) * QC],
                                    start=(ct == 0), stop=(ct == CT - 1))
                            dest = qT[colt] if colt < 3 else kT[colt - 3]
                            nc.vector.tensor_scalar_add(
                                dest[:, j * QC:(j + 1) * QC], qps[:],
                                bqk[:, colt:colt + 1])

                    for j in range(NCH):
                        qk_cols(j, (3, 4, 5))
                    for nt in range(NT):
                        vps = vpsp.tile([P, CQK], F32, name="vps", tag="vps")
                        for ct in range(CT):
                            nc.tensor.matmul(
                                vps[:],
                                xt_sb[ct][:, nt * P:(nt + 1) * P],
                                w_sb[ct][:, 2 * CQK:3 * CQK],
                                start=(ct == 0), stop=False)
                        nc.tensor.matmul(
                            vps[:], ones[0:1, 0:P], bv[:], start=False,
                            stop=True)
                        for h in range(HPC):
                            nc.vector.tensor_copy(
                                v_sb[:, (nt * HPC + h) * VW:
                                     (nt * HPC + h) * VW + HD],
                                vps[:, h * HD:(h + 1) * HD])
                    for j in range(NCH):
                        qk_cols(j, (0, 1, 2))

                # ------------- phase 2: attention -------------
                with (
                    tc.tile_pool(name="psa", bufs=1, space="PSUM") as psap,
                    tc.tile_pool(name="psb", bufs=1, space="PSUM") as psbp,
                    tc.tile_pool(name="avps", bufs=1, space="PSUM") as avp,
                ):
                    if True:
                        psA = psap.tile([P, 4 * QC], F32, name="psA", tag="psA")
                        psB = psbp.tile([P, 2 * QC], F32, name="psB", tag="psB")
                        av = avp.tile([P, 2 * QC], F32, name="av", tag="av")
                        GOFF = [sum(GROUPS[:g]) for g in range(len(GROUPS))]

                        for j in range(NCH):
                            qsl = slice(j * QC, (j + 1) * QC)
                            for p in range(3):

                                def scores(g):
                                    ps = psA if g % 2 == 0 else psB
                                    for i in range(GROUPS[g]):
                                        kt = GOFF[g] + i
                                        for par in range(2):
                                            rows = slice(par * HD,
                                                         (par + 1) * HD)
                                            nc.tensor.matmul(
                                                ps[:, (2 * i + par) * QC:
                                                   (2 * i + par + 1) * QC],
                                                kT[p][rows,
                                                      kt * P:(kt + 1) * P],
                                                qT[p][rows, qsl],
                                                start=True, stop=True)

                                def expg(g):
                                    ps = psA if g % 2 == 0 else psB
                                    ex = ex_pool.tile(
                                        [P, 2 * 2 * QC], MMDT,
                                        name="ex", tag="ex")
                                    nc.scalar.activation(
                                        ex[:, 0:2 * GROUPS[g] * QC],
                                        ps[:, 0:2 * GROUPS[g] * QC], EXP)
                                    return ex

                                def avg(g, ex):
                                    for i in range(GROUPS[g]):
                                        kt = GOFF[g] + i
                                        for par in range(2):
                                            h = 2 * p + par
                                            nc.tensor.matmul(
                                                av[0:VW, par * QC:
                                                   (par + 1) * QC],
                                                v_sb[:, (kt * HPC + h) * VW:
                                                     (kt * HPC + h + 1) * VW],
                                                ex[:, (2 * i + par) * QC:
                                                   (2 * i + par + 1) * QC],
                                                start=(kt == 0),
                                                stop=(kt == NT - 1))

                                exs = {}
                                scores(0)
                                exs[0] = expg(0)
                                scores(1)
                                exs[1] = expg(1)
                                for g in range(2, len(GROUPS)):
                                    avg(g - 2, exs.pop(g - 2))
                                    scores(g)
                                    exs[g] = expg(g)
                                avg(len(GROUPS) - 2, exs.pop(len(GROUPS) - 2))
                                avg(len(GROUPS) - 1, exs.pop(len(GROUPS) - 1))

                                # drain av to SBUF (frees banks for the next
                                # pair), then normalize from SBUF
                                avc = avs_pool.tile([VW, 2 * QC], F32,
                                                    name="avc", tag="avc")
                                nc.vector.tensor_copy(avc[:], av[0:VW, :])
                                rec = rec_pool.tile([1, 2 * QC], F32,
                                                    name="rec", tag="rec")
                                nc.vector.reciprocal(
                                    rec[:], avc[HD:HD + 1, :])
                                bc = bc_pool.tile([HD, 2 * QC], F32,
                                                  name="bc", tag="bc")
                                if bcast_mode == "pool":
                                    nc.gpsimd.partition_broadcast(
                                        bc[:], rec[:])
                                    for par in range(2):
                                        rows = slice(par * HD, (par + 1) * HD)
                                        csl = slice(par * QC, (par + 1) * QC)
                                        nc.vector.tensor_mul(
                                            o_sb[p][rows, qsl],
                                            avc[0:HD, csl], bc[:, csl])
                                else:
                                    for par in range(2):
                                        rows = slice(par * HD, (par + 1) * HD)
                                        csl = slice(par * QC, (par + 1) * QC)
                                        nc.vector.tensor_mul(
                                            o_sb[p][rows, qsl],
                                            avc[0:HD, csl],
                                            rec[0:1, csl].partition_broadcast(
                                                HD))



            # ---------------- phase 3: proj ----------------
            with (
                tc.tile_pool(name="outsb", bufs=3) as out_pool,
                tc.tile_pool(name="pps", bufs=2, space="PSUM") as proj_ps,
                tc.tile_pool(name="pps2", bufs=2, space="PSUM") as proj_ps2,
            ):
                for t in range(N // P):
                    osb = out_pool.tile([P, C], MMDT, name="osb", tag="osb")
                    for n0, nw, pool in ((0, QC, proj_ps),
                                         (QC, C - QC, proj_ps2)):
                        pps = pool.tile([P, nw], F32, name="pp", tag="pp")
                        for p in range(3):
                            nc.tensor.matmul(
                                pps[:],
                                o_sb[p][:, t * P:(t + 1) * P],
                                wp_sb[p][:, n0:n0 + nw],
                                start=(p == 0), stop=(p == 2))
                        nc.scalar.activation(osb[:, n0:n0 + nw], pps[:], COPY)
                    nc.sync.dma_start(out_d[t * P:(t + 1) * P, :], osb[:])

    nc.compile()
    return nc


def _get_program(mm_dt=BF16, bcast_mode="pool"):
    key = (str(mm_dt), bcast_mode)
    if key not in _CACHE:
        _CACHE[key] = build_program(mm_dt, bcast_mode)
    return _CACHE[key]


def make_in_maps(x, w_qkv, b_qkv, w_proj):
    import ml_dtypes
    bf = ml_dtypes.bfloat16
    x = np.ascontiguousarray(x, np.float32)
    w_qkv = np.asarray(w_qkv, np.float32)
    b_qkv = np.asarray(b_qkv, np.float32)
    w_proj = np.asarray(w_proj, np.float32)
    in_maps = []
    xt_b = [np.ascontiguousarray(x[b].T).astype(bf) for b in range(B)]
    for c in range(8):
        b, hg = divmod(c, 2)
        hsl = slice(hg * HPC * HD, (hg + 1) * HPC * HD)
        wq = w_qkv[:, 0:C][:, hsl] * SCALE
        wk = w_qkv[:, C:2 * C][:, hsl]
        wv = w_qkv[:, 2 * C:3 * C][:, hsl]
        w_in = np.ascontiguousarray(
            np.concatenate([wq, wk, wv], axis=1)).astype(bf)
        bq = b_qkv[0:C][hsl] * SCALE
        bk = b_qkv[C:2 * C][hsl]
        bvv = b_qkv[2 * C:3 * C][hsl]
        bqk_in = np.ascontiguousarray(
            np.concatenate([bq, bk]).reshape(CT, P).T)
        wp_in = np.ascontiguousarray(w_proj[hsl, :]).astype(bf)
        in_maps.append({
            "xt": xt_b[b],
            "w": w_in,
            "wp": wp_in,
            "bqk": bqk_in,
            "bv": np.ascontiguousarray(bvv.reshape(1, CQK)).astype(bf),
        })
    return in_maps


def run(x, w_qkv, b_qkv, w_proj, b_proj, mm_dt=BF16, bcast_mode="pool",
        **run_kwargs):
    nc = _get_program(mm_dt, bcast_mode)
    in_maps = make_in_maps(x, w_qkv, b_qkv, w_proj)
    res = bass_utils.run_bass_kernel_spmd(
        nc, in_maps, core_ids=list(range(8)), **run_kwargs)
    y = np.empty((B, N, C), np.float32)
    for b in range(B):
        y[b] = (np.asarray(res.results[2 * b]["out"], np.float32)
                + np.asarray(res.results[2 * b + 1]["out"], np.float32))
    y += np.asarray(b_proj, np.float32)
    return y, res


def kernel(x, w_qkv, b_qkv, w_proj, b_proj):
    y, _ = run(x, w_qkv, b_qkv, w_proj, b_proj)
    return y


# revision 4
# speedup vs baseline: 1.0018x; 1.0018x over previous
"""Multi-head attention block on 8 Trainium2 NeuronCores — v3.

Problem: B=4, N=2048, C=768, H=12, HD=64 (f32).
  qkv = x @ w_qkv + b_qkv ; attn = softmax(q*k^T/8) ; out = (attn@v) @ w_proj + b_proj

Sharding: data-parallel over batch (4) x tensor-parallel over heads (2 groups
of 6 heads). Core c handles batch c//2, head-group c%2; the host sums the two
head-group partials per batch in f32 and adds b_proj.

v3 design (on top of v2): row-tiled score PAIRS. The two heads of a pair sit
at PE row groups 0-63 / 64-127 (tile_position auto-derived from the lhsT/out
base partitions), so their K=64 score matmuls execute CONCURRENTLY in the PE
array — ~2x on the scores phase. Attention processes a head PAIR per stream:
  - scores for 3 kt x 2 heads accumulate into a 3-bank psum tile
    [128, 3072] (pair-interleaved 512-col tiles), exp'd by ONE Act op.
  - AV accumulates both heads into a 2-bank [128, 1024] psum tile.
  - av is drained to SBUF immediately (frees the banks for the next pair),
    then normalized from SBUF: exact reciprocal (custom-DVE ops corrupt on
    HW) + broadcast + multiply.
q^T compute for chunk j+1 is emitted inside attention(j) to keep the PE fed
while Act drains the exp backlog.
"""

import numpy as np

from concourse import bacc, bass, bass_utils, tile
from concourse import mybir

B, N, C, H, HD = 4, 2048, 768, 12, 64
SCALE = HD ** -0.5
P = 128
QC = 512              # q-chunk (free dim per matmul / psum bank)
NT = N // P           # 16 key tiles
CT = C // P           # 6 contraction tiles over C
NCH = N // QC         # 4 q-chunks
HPC = 6               # heads per core
CQK = HPC * HD        # 384
VW = 65               # V columns per head incl. ones column
F32 = mybir.dt.float32
BF16 = mybir.dt.bfloat16
EXP = mybir.ActivationFunctionType.Exp
COPY = mybir.ActivationFunctionType.Copy

# kt-group sizes per (j, pair): each group's 2*g score tiles (both heads,
# pair-interleaved) go to one psum tile and one Act exp. The (2,1) alternation
# maps 2-kt groups to the 4-bank psA and 1-kt groups to the 2-bank psB
# (4 + 2 + 2 av = 8 banks).
GROUPS = (2, 1, 2, 1, 2, 1, 2, 1, 2, 1, 1)

_CACHE = {}


def build_program(mm_dt=BF16, bcast_mode="pool"):
    MMDT = mm_dt
    nc = bacc.Bacc("TRN2", target_bir_lowering=False, debug=False, num_devices=8)

    xt_d = nc.dram_tensor("xt", [C, N], MMDT, kind="ExternalInput")
    w_d = nc.dram_tensor("w", [C, 3 * CQK], MMDT, kind="ExternalInput")
    wp_d = nc.dram_tensor("wp", [CQK, C], MMDT, kind="ExternalInput")
    bqk_d = nc.dram_tensor("bqk", [P, CT], F32, kind="ExternalInput")
    bv_d = nc.dram_tensor("bv", [1, CQK], MMDT, kind="ExternalInput")
    out_d = nc.dram_tensor("out", [N, C], MMDT, kind="ExternalOutput")

    with tile.TileContext(nc) as tc, nc.allow_low_precision(
            reason="bf16 matmuls; host accumulates partials in f32"):
        with (
            tc.tile_pool(name="const", bufs=1) as cpool,
            tc.tile_pool(name="persist", bufs=1) as pp,
        ):
            ones = cpool.tile([1, P], MMDT, name="ones", tag="ones")
            nc.gpsimd.memset(ones[:], 1.0)
            ones_f = cpool.tile([P, NT * HPC], F32, name="ones_f", tag="ones_f")
            nc.gpsimd.memset(ones_f[:], 1.0)
            bqk = cpool.tile([P, CT], F32, name="bqk", tag="bqk")
            nc.sync.dma_start(bqk[:], bqk_d[:])
            bv = cpool.tile([1, CQK], MMDT, name="bv", tag="bv")
            nc.sync.dma_start(bv[:], bv_d[:])

            # DMA order matters: the first k^T matmul needs the w_k slices
            # and x^T chunk j=0, so those are emitted first (~12 parallel
            # queue transfers) instead of burying them behind the full x load.
            xt_sb = [pp.tile([P, N], MMDT, name=f"xt{ct}", tag=f"xt{ct}")
                     for ct in range(CT)]
            w_sb = [pp.tile([P, 3 * CQK], MMDT, name=f"w{ct}", tag=f"w{ct}")
                    for ct in range(CT)]
            for ct in range(CT):  # w_k
                nc.sync.dma_start(
                    w_sb[ct][:, CQK:2 * CQK],
                    w_d[ct * P:(ct + 1) * P, CQK:2 * CQK])
            for j in range(NCH):
                for ct in range(CT):
                    nc.sync.dma_start(
                        xt_sb[ct][:, j * QC:(j + 1) * QC],
                        xt_d[ct * P:(ct + 1) * P, j * QC:(j + 1) * QC])
            for h in (2, 0):  # w_v then w_q
                for ct in range(CT):
                    nc.sync.dma_start(
                        w_sb[ct][:, h * CQK:(h + 1) * CQK],
                        w_d[ct * P:(ct + 1) * P, h * CQK:(h + 1) * CQK])
            wp_sb = []
            for p in range(3):
                t = pp.tile([P, C], MMDT, name=f"wp{p}", tag=f"wp{p}")
                nc.sync.dma_start(t[:], wp_d[p * P:(p + 1) * P, :])
                wp_sb.append(t)

            qT = [pp.tile([P, N], MMDT, name=f"q{p}", tag=f"q{p}") for p in range(3)]
            kT = [pp.tile([P, N], MMDT, name=f"k{p}", tag=f"k{p}") for p in range(3)]
            v_sb = pp.tile([P, NT * HPC * VW], MMDT, name="v", tag="v")
            v_view = v_sb[:].rearrange("p (t w) -> p t w", w=VW)
            nc.vector.tensor_copy(
                v_view[:, :, HD:HD + 1],
                ones_f[:, 0:NT * HPC].rearrange("p (t w) -> p t w", w=1))
            # o^T packed per head-pair: rows 0-63 head 2p, 64-127 head 2p+1
            o_sb = [pp.tile([P, N], MMDT, name=f"o{p}", tag=f"o{p}") for p in range(3)]

            # ---------------- phase 1: k^T, v, q^T ----------------
            with (
                tc.tile_pool(name="exsb", bufs=3) as ex_pool,
                tc.tile_pool(name="avsb", bufs=2) as avs_pool,
                tc.tile_pool(name="recs", bufs=2) as rec_pool,
                tc.tile_pool(name="bcs", bufs=2) as bc_pool,
            ):
                with (
                    tc.tile_pool(name="qkps", bufs=3, space="PSUM") as qkps,
                    tc.tile_pool(name="vps", bufs=2, space="PSUM") as vpsp,
                ):
                    def qk_cols(j, colts):
                        for colt in colts:
                            qps = qkps.tile([P, QC], F32, name="qkp", tag="qkp")
                            for ct in range(CT):
                                nc.tensor.matmul(
                                    qps[:],
                                    w_sb[ct][:, colt * P:(colt + 1) * P],
                                    xt_sb[ct][:, j * QC:(j + 1) * QC],
                                    start=(ct == 0), stop=(ct == CT - 1))
                            dest = qT[colt] if colt < 3 else kT[colt - 3]
                            nc.vector.tensor_scalar_add(
                                dest[:, j * QC:(j + 1) * QC], qps[:],
                                bqk[:, colt:colt + 1])

                    for j in range(NCH):
                        qk_cols(j, (3, 4, 5))
                    for nt in range(NT):
                        vps = vpsp.tile([P, CQK], F32, name="vps", tag="vps")
                        for ct in range(CT):
                            nc.tensor.matmul(
                                vps[:],
                                xt_sb[ct][:, nt * P:(nt + 1) * P],
                                w_sb[ct][:, 2 * CQK:3 * CQK],
                                start=(ct == 0), stop=False)
                        nc.tensor.matmul(
                            vps[:], ones[0:1, 0:P], bv[:], start=False,
                            stop=True)
                        for h in range(HPC):
                            nc.vector.tensor_copy(
                                v_sb[:, (nt * HPC + h) * VW:
                                     (nt * HPC + h) * VW + HD],
                                vps[:, h * HD:(h + 1) * HD])
                    for j in range(NCH):
                        qk_cols(j, (0, 1, 2))

                # ------------- phase 2: attention -------------
                # One flat software pipeline across all (j, p, g) stages:
                # scores/exp for stage i run alongside AV for stage i-2, so
                # the pipeline never drains at (j, p) boundaries.
                with (
                    tc.tile_pool(name="psa", bufs=1, space="PSUM") as psap,
                    tc.tile_pool(name="psb", bufs=1, space="PSUM") as psbp,
                    tc.tile_pool(name="avps", bufs=1, space="PSUM") as avp,
                ):
                    psA = psap.tile([P, 4 * QC], F32, name="psA", tag="psA")
                    psB = psbp.tile([P, 2 * QC], F32, name="psB", tag="psB")
                    av = avp.tile([P, 2 * QC], F32, name="av", tag="av")
                    GOFF = [sum(GROUPS[:g]) for g in range(len(GROUPS))]
                    NG = len(GROUPS)

                    def scores(j, p, g):
                        ps = psA if g % 2 == 0 else psB
                        qsl = slice(j * QC, (j + 1) * QC)
                        for i in range(GROUPS[g]):
                            kt = GOFF[g] + i
                            for par in range(2):
                                rows = slice(par * HD, (par + 1) * HD)
                                nc.tensor.matmul(
                                    ps[:, (2 * i + par) * QC:
                                       (2 * i + par + 1) * QC],
                                    kT[p][rows, kt * P:(kt + 1) * P],
                                    qT[p][rows, qsl],
                                    start=True, stop=True)

                    def expg(j, p, g):
                        ps = psA if g % 2 == 0 else psB
                        ex = ex_pool.tile([P, 2 * 2 * QC], MMDT,
                                          name="ex", tag="ex")
                        nc.scalar.activation(
                            ex[:, 0:2 * GROUPS[g] * QC],
                            ps[:, 0:2 * GROUPS[g] * QC], EXP)
                        return ex

                    def avg(j, p, g, ex):
                        for i in range(GROUPS[g]):
                            kt = GOFF[g] + i
                            for par in range(2):
                                h = 2 * p + par
                                nc.tensor.matmul(
                                    av[0:VW, par * QC:(par + 1) * QC],
                                    v_sb[:, (kt * HPC + h) * VW:
                                         (kt * HPC + h + 1) * VW],
                                    ex[:, (2 * i + par) * QC:
                                       (2 * i + par + 1) * QC],
                                    start=(kt == 0), stop=(kt == NT - 1))
                        if g == NG - 1:
                            norm(j, p)

                    def norm(j, p):
                        # drain av to SBUF (frees banks for the next pair),
                        # then normalize from SBUF
                        qsl = slice(j * QC, (j + 1) * QC)
                        avc = avs_pool.tile([VW, 2 * QC], F32,
                                            name="avc", tag="avc")
                        nc.vector.tensor_copy(avc[:], av[0:VW, :])
                        rec = rec_pool.tile([1, 2 * QC], F32,
                                            name="rec", tag="rec")
                        nc.vector.reciprocal(rec[:], avc[HD:HD + 1, :])
                        bc = bc_pool.tile([HD, 2 * QC], F32,
                                          name="bc", tag="bc")
                        nc.gpsimd.partition_broadcast(bc[:], rec[:])
                        for par in range(2):
                            rows = slice(par * HD, (par + 1) * HD)
                            csl = slice(par * QC, (par + 1) * QC)
                            nc.vector.tensor_mul(
                                o_sb[p][rows, qsl],
                                avc[0:HD, csl], bc[:, csl])

                    stages = [(j, p, g) for j in range(NCH)
                              for p in range(3) for g in range(NG)]
                    exs = {}
                    for i, (j, p, g) in enumerate(stages):
                        scores(j, p, g)
                        exs[i] = expg(j, p, g)
                        if i >= 2:
                            avg(*stages[i - 2], exs.pop(i - 2))
                    n = len(stages)
                    avg(*stages[n - 2], exs.pop(n - 2))
                    avg(*stages[n - 1], exs.pop(n - 1))



            # ---------------- phase 3: proj ----------------
            with (
                tc.tile_pool(name="outsb", bufs=3) as out_pool,
                tc.tile_pool(name="pps", bufs=2, space="PSUM") as proj_ps,
                tc.tile_pool(name="pps2", bufs=2, space="PSUM") as proj_ps2,
            ):
                for t in range(N // P):
                    osb = out_pool.tile([P, C], MMDT, name="osb", tag="osb")
                    for n0, nw, pool in ((0, QC, proj_ps),
                                         (QC, C - QC, proj_ps2)):
                        pps = pool.tile([P, nw], F32, name="pp", tag="pp")
                        for p in range(3):
                            nc.tensor.matmul(
                                pps[:],
                                o_sb[p][:, t * P:(t + 1) * P],
                                wp_sb[p][:, n0:n0 + nw],
                                start=(p == 0), stop=(p == 2))
                        nc.scalar.activation(osb[:, n0:n0 + nw], pps[:], COPY)
                    nc.sync.dma_start(out_d[t * P:(t + 1) * P, :], osb[:])

    nc.compile()
    return nc


def _get_program(mm_dt=BF16, bcast_mode="pool"):
    key = (str(mm_dt), bcast_mode)
    if key not in _CACHE:
        _CACHE[key] = build_program(mm_dt, bcast_mode)
    return _CACHE[key]


def make_in_maps(x, w_qkv, b_qkv, w_proj):
    import ml_dtypes
    bf = ml_dtypes.bfloat16
    x = np.ascontiguousarray(x, np.float32)
    w_qkv = np.asarray(w_qkv, np.float32)
    b_qkv = np.asarray(b_qkv, np.float32)
    w_proj = np.asarray(w_proj, np.float32)
    in_maps = []
    xt_b = [np.ascontiguousarray(x[b].T).astype(bf) for b in range(B)]
    for c in range(8):
        b, hg = divmod(c, 2)
        hsl = slice(hg * HPC * HD, (hg + 1) * HPC * HD)
        wq = w_qkv[:, 0:C][:, hsl] * SCALE
        wk = w_qkv[:, C:2 * C][:, hsl]
        wv = w_qkv[:, 2 * C:3 * C][:, hsl]
        w_in = np.ascontiguousarray(
            np.concatenate([wq, wk, wv], axis=1)).astype(bf)
        bq = b_qkv[0:C][hsl] * SCALE
        bk = b_qkv[C:2 * C][hsl]
        bvv = b_qkv[2 * C:3 * C][hsl]
        bqk_in = np.ascontiguousarray(
            np.concatenate([bq, bk]).reshape(CT, P).T)
        wp_in = np.ascontiguousarray(w_proj[hsl, :]).astype(bf)
        in_maps.append({
            "xt": xt_b[b],
            "w": w_in,
            "wp": wp_in,
            "bqk": bqk_in,
            "bv": np.ascontiguousarray(bvv.reshape(1, CQK)).astype(bf),
        })
    return in_maps


def run(x, w_qkv, b_qkv, w_proj, b_proj, mm_dt=BF16, bcast_mode="pool",
        **run_kwargs):
    nc = _get_program(mm_dt, bcast_mode)
    in_maps = make_in_maps(x, w_qkv, b_qkv, w_proj)
    res = bass_utils.run_bass_kernel_spmd(
        nc, in_maps, core_ids=list(range(8)), **run_kwargs)
    y = np.empty((B, N, C), np.float32)
    for b in range(B):
        y[b] = (np.asarray(res.results[2 * b]["out"], np.float32)
                + np.asarray(res.results[2 * b + 1]["out"], np.float32))
    y += np.asarray(b_proj, np.float32)
    return y, res


def kernel(x, w_qkv, b_qkv, w_proj, b_proj):
    y, _ = run(x, w_qkv, b_qkv, w_proj, b_proj)
    return y


# revision 6
# speedup vs baseline: 1.0182x; 1.0163x over previous
"""Multi-head attention block on 8 Trainium2 NeuronCores — v9.

Problem: B=4, N=2048, C=768, H=12, HD=64 (f32).
  qkv = x @ w_qkv + b_qkv ; attn = softmax(q*k^T/8) ; out = (attn@v) @ w_proj + b_proj

Sharding: data-parallel over batch (4) x tensor-parallel over heads (2 groups
of 6 heads); host sums the two head-group partials per batch in f32.

v9 (on top of the 292us flat-pipeline kernel): full phase fusion with the
proven (2,1) exp groups. The enabler is a SINGLE-bank attn@V accumulator:
exp tiles persist in SBUF for a whole pair (the two pools exA/exB hold two
pairs' worth), and AV runs as two sequential passes (head 0 over all 16 kt,
drain, then head 1 re-reading the same exp tiles). That frees one PSUM bank
(psA 4 + psB 2 + av 1 + qkv-filler 1 = 8), so the ENTIRE phase-1 (v, and all
k/q columns beyond a ~6us prefix) is emitted as deadline-scheduled PE filler
inside the Act-bound attention stream. AV for pair X-1 is chunk-interleaved
through pair X's stages, so the av drain and normalization overlap scores.
"""

import numpy as np

from concourse import bacc, bass, bass_utils, tile
from concourse import mybir

B, N, C, H, HD = 4, 2048, 768, 12, 64
SCALE = HD ** -0.5
P = 128
QC = 512              # q-chunk (free dim per matmul / psum bank)
NT = N // P           # 16 key tiles
CT = C // P           # 6 contraction tiles over C
NCH = N // QC         # 4 q-chunks
HPC = 6               # heads per core
CQK = HPC * HD        # 384
VW = 65               # V columns per head incl. ones column
F32 = mybir.dt.float32
BF16 = mybir.dt.bfloat16
EXP = mybir.ActivationFunctionType.Exp
COPY = mybir.ActivationFunctionType.Copy

GROUPS = (2, 1, 2, 1, 2, 1, 2, 1, 2, 1, 1)
GOFF = [sum(GROUPS[:g]) for g in range(len(GROUPS))]
NG = len(GROUPS)
# avg chunk per local stage: h0 kts over stages 0-4, h1 kts over 5-9
AVG_CHUNKS = [(0, (0, 1, 2)), (0, (3, 4, 5)), (0, (6, 7, 8)),
              (0, (9, 10, 11)), (0, (12, 13, 14, 15)),
              (1, (0, 1, 2)), (1, (3, 4, 5)), (1, (6, 7, 8)),
              (1, (9, 10, 11)), (1, (12, 13, 14, 15)), (2, ())]

_CACHE = {}


def build_program(mm_dt=BF16):
    MMDT = mm_dt
    nc = bacc.Bacc("TRN2", target_bir_lowering=False, debug=False, num_devices=8)

    xt_d = nc.dram_tensor("xt", [C, N], MMDT, kind="ExternalInput")
    w_d = nc.dram_tensor("w", [C, 3 * CQK], MMDT, kind="ExternalInput")
    wp_d = nc.dram_tensor("wp", [CQK, C], MMDT, kind="ExternalInput")
    bqk_d = nc.dram_tensor("bqk", [P, CT], F32, kind="ExternalInput")
    bv_d = nc.dram_tensor("bv", [1, CQK], MMDT, kind="ExternalInput")
    out_d = nc.dram_tensor("out", [N, C], MMDT, kind="ExternalOutput")

    with tile.TileContext(nc) as tc, nc.allow_low_precision(
            reason="bf16 matmuls; host accumulates partials in f32"):
        with (
            tc.tile_pool(name="const", bufs=1) as cpool,
            tc.tile_pool(name="persist", bufs=1) as pp,
        ):
            ones = cpool.tile([1, P], MMDT, name="ones", tag="ones")
            nc.gpsimd.memset(ones[:], 1.0)
            ones_f = cpool.tile([P, NT * HPC], F32, name="ones_f", tag="ones_f")
            nc.gpsimd.memset(ones_f[:], 1.0)
            bqk = cpool.tile([P, CT], F32, name="bqk", tag="bqk")
            nc.sync.dma_start(bqk[:], bqk_d[:])
            bv = cpool.tile([1, CQK], MMDT, name="bv", tag="bv")
            nc.sync.dma_start(bv[:], bv_d[:])

            xt_sb = [pp.tile([P, N], MMDT, name=f"xt{ct}", tag=f"xt{ct}")
                     for ct in range(CT)]
            w_sb = [pp.tile([P, 3 * CQK], MMDT, name=f"w{ct}", tag=f"w{ct}")
                    for ct in range(CT)]
            # w_k + xt j0 first (prefix needs them), then the rest
            for ct in range(CT):
                nc.scalar.dma_start(
                    w_sb[ct][:, CQK:2 * CQK],
                    w_d[ct * P:(ct + 1) * P, CQK:2 * CQK])
                nc.sync.dma_start(
                    xt_sb[ct][:, 0:QC], xt_d[ct * P:(ct + 1) * P, 0:QC])
            for ct in range(CT):  # w_q needed by the prefix q(j0,c0)
                nc.scalar.dma_start(
                    w_sb[ct][:, 0:CQK], w_d[ct * P:(ct + 1) * P, 0:CQK])
            for j in range(1, NCH):
                for ct in range(CT):
                    nc.gpsimd.dma_start(
                        xt_sb[ct][:, j * QC:(j + 1) * QC],
                        xt_d[ct * P:(ct + 1) * P, j * QC:(j + 1) * QC])
            for ct in range(CT):  # w_v
                nc.gpsimd.dma_start(
                    w_sb[ct][:, 2 * CQK:3 * CQK],
                    w_d[ct * P:(ct + 1) * P, 2 * CQK:3 * CQK])
            wp_sb = []
            for p in range(3):
                t = pp.tile([P, C], MMDT, name=f"wp{p}", tag=f"wp{p}")
                nc.gpsimd.dma_start(t[:], wp_d[p * P:(p + 1) * P, :])
                wp_sb.append(t)

            qT = [pp.tile([P, N], MMDT, name=f"q{p}", tag=f"q{p}") for p in range(3)]
            kT = [pp.tile([P, N], MMDT, name=f"k{p}", tag=f"k{p}") for p in range(3)]
            v_sb = pp.tile([P, NT * HPC * VW], MMDT, name="v", tag="v")
            v_view = v_sb[:].rearrange("p (t w) -> p t w", w=VW)
            nc.vector.tensor_copy(
                v_view[:, :, HD:HD + 1],
                ones_f[:, 0:NT * HPC].rearrange("p (t w) -> p t w", w=1))
            o_sb = [pp.tile([P, N], MMDT, name=f"o{p}", tag=f"o{p}") for p in range(3)]

            pairs = [(j, p) for j in range(NCH) for p in range(3)]

            with (
                tc.tile_pool(name="exA", bufs=11) as exA_pool,
                tc.tile_pool(name="exB", bufs=13) as exB_pool,
                tc.tile_pool(name="avsb", bufs=4) as avs_pool,
                tc.tile_pool(name="recs", bufs=2) as rec_pool,
                tc.tile_pool(name="bcs", bufs=2) as bc_pool,
            ):
                with (
                    tc.tile_pool(name="psa", bufs=1, space="PSUM") as psap,
                    tc.tile_pool(name="psb", bufs=1, space="PSUM") as psbp,
                    tc.tile_pool(name="avps", bufs=1, space="PSUM") as avp,
                    tc.tile_pool(name="fillps", bufs=1, space="PSUM") as fillp,
                ):
                    psA = psap.tile([P, 4 * QC], F32, name="psA", tag="psA")
                    psB = psbp.tile([P, 2 * QC], F32, name="psB", tag="psB")
                    av = avp.tile([VW, QC], F32, name="av", tag="av")

                    def qk_col(j, colt):
                        qps = fillp.tile([P, QC], F32, name="fillt",
                                         tag="fillt")
                        for ct in range(CT):
                            nc.tensor.matmul(
                                qps[:],
                                w_sb[ct][:, colt * P:(colt + 1) * P],
                                xt_sb[ct][:, j * QC:(j + 1) * QC],
                                start=(ct == 0), stop=(ct == CT - 1))
                        dest = qT[colt] if colt < 3 else kT[colt - 3]
                        nc.vector.tensor_scalar_add(
                            dest[:, j * QC:(j + 1) * QC], qps[:],
                            bqk[:, colt:colt + 1])

                    def v_group(nt):
                        vps = fillp.tile([P, QC], F32, name="fillt",
                                         tag="fillt")[:, 0:CQK]
                        for ct in range(CT):
                            nc.tensor.matmul(
                                vps[:],
                                xt_sb[ct][:, nt * P:(nt + 1) * P],
                                w_sb[ct][:, 2 * CQK:3 * CQK],
                                start=(ct == 0), stop=False)
                        nc.tensor.matmul(
                            vps[:], ones[0:1, 0:P], bv[:], start=False,
                            stop=True)
                        dst = v_sb[:, nt * HPC * VW:(nt + 1) * HPC * VW]
                        nc.vector.tensor_copy(
                            dst.rearrange("p (h w) -> p h w", w=VW)[:, :, 0:HD],
                            vps[:].rearrange("p (h d) -> p h d", d=HD))

                    def scores(X, g):
                        j, p = pairs[X]
                        ps = psA if g % 2 == 0 else psB
                        qsl = slice(j * QC, (j + 1) * QC)
                        for i in range(GROUPS[g]):
                            kt = GOFF[g] + i
                            for par in range(2):
                                rows = slice(par * HD, (par + 1) * HD)
                                nc.tensor.matmul(
                                    ps[:, (2 * i + par) * QC:
                                       (2 * i + par + 1) * QC],
                                    kT[p][rows, kt * P:(kt + 1) * P],
                                    qT[p][rows, qsl],
                                    start=True, stop=True)

                    def expg(X, g):
                        ps = psA if g % 2 == 0 else psB
                        w = 2 * GROUPS[g] * QC
                        pool = exA_pool if GROUPS[g] == 2 else exB_pool
                        ex = pool.tile([P, w], MMDT, name="ex", tag="ex")
                        nc.scalar.activation(ex[:, 0:w], ps[:, 0:w], EXP)
                        return ex

                    # exp tiles of pair X, keyed by kt: (ex, col_of_h0)
                    def ex_slices(exmap, kt):
                        g = next(g for g in range(NG)
                                 if GOFF[g] <= kt < GOFF[g] + GROUPS[g])
                        i = kt - GOFF[g]
                        return exmap[g], 2 * i

                    avcs = {}

                    def avg_chunk(X, par, kts, exmap, acc=None):
                        acc = av if acc is None else acc
                        j, p = pairs[X]
                        h = 2 * p + par
                        for kt in kts:
                            ex, i2 = ex_slices(exmap, kt)
                            nc.tensor.matmul(
                                acc[:, :],
                                v_sb[:, (kt * HPC + h) * VW:
                                     (kt * HPC + h + 1) * VW],
                                ex[:, (i2 + par) * QC:(i2 + par + 1) * QC],
                                start=(kt == 0), stop=(kt == NT - 1))
                        if kts and kts[-1] == NT - 1:
                            avc = avs_pool.tile([VW, QC], F32,
                                                name="avc", tag="avc")
                            nc.vector.tensor_copy(avc[:], acc[:])
                            avcs[(X, par)] = avc

                    def norm(X):
                        j, p = pairs[X]
                        qsl = slice(j * QC, (j + 1) * QC)
                        rec = rec_pool.tile([1, 2 * QC], F32, name="rec",
                                            tag="rec")
                        bc = bc_pool.tile([HD, 2 * QC], F32, name="bc",
                                          tag="bc")
                        for par in range(2):
                            avc = avcs[(X, par)]
                            csl = slice(par * QC, (par + 1) * QC)
                            nc.vector.reciprocal(
                                rec[0:1, csl], avc[HD:HD + 1, :])
                            nc.gpsimd.partition_broadcast(
                                bc[:, csl], rec[0:1, csl])
                        for par in range(2):
                            avc = avcs.pop((X, par))
                            rows = slice(par * HD, (par + 1) * HD)
                            csl = slice(par * QC, (par + 1) * QC)
                            nc.vector.tensor_mul(
                                o_sb[p][rows, qsl], avc[0:HD, :], bc[:, csl])

                    # filler schedule (global stage index -> emitters); all
                    # deadlines precede first consumption (see design notes)
                    fill = {}

                    def add_fill(s, fn, *args):
                        fill.setdefault(s, []).append((fn,) + args)

                    add_fill(0, qk_col, 1, 3)     # kT[0] j1, by stage 2
                    add_fill(1, qk_col, 2, 3)     # by stage 5
                    add_fill(2, qk_col, 3, 3)     # by stage 8
                    for i, jj in enumerate((0, 1, 2, 3)):   # kT[1], by 11..19
                        add_fill(6 + 2 * i, qk_col, jj, 4)
                    for i, jj in enumerate((0, 1, 2, 3)):   # kT[2], by 22..30
                        add_fill(16 + 3 * i, qk_col, jj, 5)
                    # v(kt) needed by stage 11+kt//3 (avg chunk of pair 0);
                    # keep it late enough that the w_v DMA has landed
                    for nt in range(8):
                        add_fill(3 + nt, v_group, nt)
                    for nt, s in zip(range(8, 16),
                                     (11, 11, 12, 12, 13, 13, 14, 15)):
                        add_fill(s, v_group, nt)
                    add_fill(9, qk_col, 0, 1)     # qT[1] j0, by stage 11
                    add_fill(18, qk_col, 0, 2)    # by 22
                    add_fill(26, qk_col, 1, 0)    # by 33
                    add_fill(30, qk_col, 1, 1)    # by 44
                    add_fill(38, qk_col, 1, 2)    # by 55
                    add_fill(50, qk_col, 2, 0)    # by 66
                    add_fill(60, qk_col, 2, 1)    # by 77
                    add_fill(70, qk_col, 2, 2)    # by 88
                    add_fill(80, qk_col, 3, 0)    # by 99
                    add_fill(90, qk_col, 3, 1)    # by 110
                    add_fill(100, qk_col, 3, 2)   # by 121

                    # prefix: kT[0] j0 + qT[0] j0
                    qk_col(0, 3)
                    qk_col(0, 0)

                    last = len(pairs) - 1
                    av2 = [None]
                    exmaps = {}
                    for X in range(len(pairs)):
                        exmaps[X] = {}
                        for g in range(NG):
                            s = X * NG + g
                            for f in fill.get(s, ()):
                                f[0](*f[1:])
                            scores(X, g)
                            exmaps[X][g] = expg(X, g)
                            if X >= 1:
                                par, kts = AVG_CHUNKS[g]
                                if par < 2:
                                    avg_chunk(X - 1, par, list(kts),
                                              exmaps[X - 1])
                                else:
                                    norm(X - 1)
                                    del exmaps[X - 1]
                            if X == last and g >= 2:
                                if av2[0] is None:
                                    av2[0] = fillp.tile(
                                        [P, QC], F32, name="fillt",
                                        tag="fillt")[0:VW, 0:QC]
                                g2 = g - 2
                                kts2 = list(range(GOFF[g2],
                                                  GOFF[g2] + GROUPS[g2]))
                                avg_chunk(last, 0, kts2, exmaps[last],
                                          acc=av2[0])
                    # drain: h0 leftovers, then h1, then norm for last pair
                    for g2 in (NG - 2, NG - 1):
                        kts2 = list(range(GOFF[g2], GOFF[g2] + GROUPS[g2]))
                        avg_chunk(last, 0, kts2, exmaps[last], acc=av2[0])
                    for par, kts in AVG_CHUNKS[5:]:
                        if par == 1:
                            avg_chunk(last, 1, list(kts), exmaps[last],
                                      acc=av2[0])
                        else:
                            norm(last)

            # ---------------- proj ----------------
            with (
                tc.tile_pool(name="outsb", bufs=3) as out_pool,
                tc.tile_pool(name="pps", bufs=2, space="PSUM") as proj_ps,
                tc.tile_pool(name="pps2", bufs=2, space="PSUM") as proj_ps2,
            ):
                for t in range(N // P):
                    osb = out_pool.tile([P, C], MMDT, name="osb", tag="osb")
                    for n0, nw, pool, drain in (
                            (0, QC, proj_ps, "act"),
                            (QC, C - QC, proj_ps2, "dve")):
                        pps = pool.tile([P, nw], F32, name="pp", tag="pp")
                        for p in range(3):
                            nc.tensor.matmul(
                                pps[:],
                                o_sb[p][:, t * P:(t + 1) * P],
                                wp_sb[p][:, n0:n0 + nw],
                                start=(p == 0), stop=(p == 2))
                        if drain == "act":
                            nc.scalar.activation(
                                osb[:, n0:n0 + nw], pps[:], COPY)
                        else:
                            nc.vector.tensor_copy(osb[:, n0:n0 + nw], pps[:])
                    nc.sync.dma_start(
                        out_d[t * P:(t + 1) * P, 0:QC], osb[:, 0:QC])
                    nc.scalar.dma_start(
                        out_d[t * P:(t + 1) * P, QC:C], osb[:, QC:C])

    nc.compile()
    return nc


def _get_program(mm_dt=BF16):
    key = str(mm_dt)
    if key not in _CACHE:
        _CACHE[key] = build_program(mm_dt)
    return _CACHE[key]


def make_in_maps(x, w_qkv, b_qkv, w_proj):
    import ml_dtypes
    bf = ml_dtypes.bfloat16
    x = np.ascontiguousarray(x, np.float32)
    w_qkv = np.asarray(w_qkv, np.float32)
    b_qkv = np.asarray(b_qkv, np.float32)
    w_proj = np.asarray(w_proj, np.float32)
    in_maps = []
    xt_b = [np.ascontiguousarray(x[b].T).astype(bf) for b in range(B)]
    for c in range(8):
        b, hg = divmod(c, 2)
        hsl = slice(hg * HPC * HD, (hg + 1) * HPC * HD)
        wq = w_qkv[:, 0:C][:, hsl] * SCALE
        wk = w_qkv[:, C:2 * C][:, hsl]
        wv = w_qkv[:, 2 * C:3 * C][:, hsl]
        w_in = np.ascontiguousarray(
            np.concatenate([wq, wk, wv], axis=1)).astype(bf)
        bq = b_qkv[0:C][hsl] * SCALE
        bk = b_qkv[C:2 * C][hsl]
        bvv = b_qkv[2 * C:3 * C][hsl]
        bqk_in = np.ascontiguousarray(
            np.concatenate([bq, bk]).reshape(CT, P).T)
        wp_in = np.ascontiguousarray(w_proj[hsl, :]).astype(bf)
        in_maps.append({
            "xt": xt_b[b],
            "w": w_in,
            "wp": wp_in,
            "bqk": bqk_in,
            "bv": np.ascontiguousarray(bvv.reshape(1, CQK)).astype(bf),
        })
    return in_maps


def run(x, w_qkv, b_qkv, w_proj, b_proj, mm_dt=BF16, **run_kwargs):
    nc = _get_program(mm_dt)
    in_maps = make_in_maps(x, w_qkv, b_qkv, w_proj)
    res = bass_utils.run_bass_kernel_spmd(
        nc, in_maps, core_ids=list(range(8)), **run_kwargs)
    y = np.empty((B, N, C), np.float32)
    for b in range(B):
        y[b] = (np.asarray(res.results[2 * b]["out"], np.float32)
                + np.asarray(res.results[2 * b + 1]["out"], np.float32))
    y += np.asarray(b_proj, np.float32)
    return y, res


def kernel(x, w_qkv, b_qkv, w_proj, b_proj):
    y, _ = run(x, w_qkv, b_qkv, w_proj, b_proj)
    return y
